# revision 3
# baseline (speedup 1.0000x reference)
"""Single-head causal self-attention on 8 Trainium2 NeuronCores — optimized.

Reference computation (per batch b):
    k = x @ Wk.T ; q = x @ Wq.T ; v = x @ Wv.T
    wei = softmax(mask(q @ k.T / sqrt(H)))
    out = wei @ v

Design (see git/transcript history for the evolution):
  - All matmul operands bf16 (PSUM accumulate fp32): bf16 stationary
    operands get Fast Weight Load and bf16 moving operands stream
    1 row/cycle.  Measured numerics: ~4e-3 rel err (tolerance 2e-2).
  - x is cast f32->bf16 INSIDE the input DMA (GpSimd SWDGE DMAs can
    convert dtypes) — no separate on-chip cast pass.
  - G = (Wq.T @ Wk) * scale precomputed once (q/k fusion halves the
    projection work).
  - Batches processed in pairs: transpose/z2 matmuls stream N=512.
  - Causal skips: the (s-chunk 1, t-chunk 0) score block is never
    computed; t-chunk 0 output only consumes s-chunk 0.
  - PSUM->SBUF evacuations are merged into single wide ops over
    multi-bank PSUM tiles and spread across DVE and ACT; pure-SBUF ops
    (masks, ones) go to GpSimd.  exp runs on ACT straight out of PSUM
    writing bf16; causal mask via affine_select on the two 128x128
    diagonal blocks only.
  - Softmax denominator via ones-columns appended to V; the final
    normalize is fused with the PSUM->SBUF copy (reciprocal on DVE,
    scaled copies split DVE/ACT).
  - PSUM plan (8 banks): TZ 3 banks (transposes as bf16, then z2),
    TV 2 banks (per-batch V), PST 1 bank (scores), PO0/PO1 1 bank each
    (output accumulators).
"""

import numpy as np

import concourse.bass as bass
import concourse.mybir as mybir
from concourse import bacc
import concourse.tile as tile
from concourse.bass_utils import run_bass_kernel_spmd
from concourse.masks import make_identity

B, T, C, H = 256, 256, 384, 384
NCORES = 8
NB = B // NCORES  # batches per core
P = 128
CC = C // P  # 3 chunks of the embedding dim
SCALE = float(H) ** -0.5
F32 = mybir.dt.float32
BF16 = mybir.dt.bfloat16

EXP = mybir.ActivationFunctionType.Exp
COPY = mybir.ActivationFunctionType.Copy
GE = mybir.AluOpType.is_ge


def build_bass(nb: int = NB):
    nc = bacc.Bacc(
        "TRN2",
        target_bir_lowering=False,
        debug=False,
        enable_asserts=False,
        num_devices=NCORES,
    )
    x_d = nc.dram_tensor("x", [nb, T, C], F32, kind="ExternalInput").ap()
    wk_d = nc.dram_tensor("Wk", [H, C], F32, kind="ExternalInput").ap()
    wq_d = nc.dram_tensor("Wq", [H, C], F32, kind="ExternalInput").ap()
    wv_d = nc.dram_tensor("Wv", [H, C], F32, kind="ExternalInput").ap()
    out_d = nc.dram_tensor("out", [nb, T, H], F32, kind="ExternalOutput").ap()

    assert nb % 2 == 0
    npair = nb // 2

    with tile.TileContext(nc) as tc:
        with (
            tc.tile_pool(name="const", bufs=1) as cpool,
            tc.tile_pool(name="sb", bufs=2) as sb,
            tc.tile_pool(name="ob", bufs=4) as obp,
            tc.tile_pool(name="pp", bufs=1, space="PSUM") as pp,
        ):
            ident = cpool.tile([P, P], BF16, name="ident")
            make_identity(nc, ident)

            class Pair:
                """Stage emitters for one batch pair; instances let pair
                k+1's transposes be issued before pair k's output matmuls
                (software pipelining across the in-order PE queue)."""

                def __init__(self, pr):
                    self.b0 = 2 * pr

                def load_transpose(self):
                    b0 = self.b0
                    # xb[p, u*768 + b*384 + c] = x[b0+b, u*128+p, c],
                    # cast f32->bf16 inside the DMA (GpSimd SWDGE)
                    xb = sb.tile([P, 1536], BF16, name="xb", tag="xb", bufs=3)
                    for u in range(2):
                        nc.gpsimd.dma_start(
                            xb[:, u * 768 : (u + 1) * 768],
                            x_d[b0 : b0 + 2, u * P : (u + 1) * P, :].rearrange(
                                "b t c -> t b c"
                            ),
                        )
                    # transpose to xT[:, cc*512 + b*256 + u*128 + t']
                    xT = sb.tile([P, 1536], BF16, name="xT", tag="xT", bufs=3)
                    for cc_ in range(CC):
                        ptx = pp.tile([P, 1024], BF16, name="ptx", tag=f"T{cc_}")
                        for b in range(2):
                            for u in range(2):
                                nc.tensor.transpose(
                                    ptx[:, b * 256 + u * P : b * 256 + (u + 1) * P],
                                    xb[:, u * 768 + b * 384 + cc_ * P : u * 768 + b * 384 + (cc_ + 1) * P],
                                    ident,
                                )
                        nc.vector.tensor_copy(
                            xT[:, cc_ * 512 : (cc_ + 1) * 512], ptx[:, :512]
                        )
                    self.xT = xT

                def z2_vst(self):
                    xT = self.xT
                    # z2[c2] = (G.T @ xT): z2[:, c2*512 + b*256 + u*128 + t']
                    z2 = sb.tile([P, 1536], BF16, name="z2", tag="z2")
                    for c2 in range(CC):
                        pz = pp.tile([P, 512], F32, name="pz", tag=f"T{c2}")
                        for c1 in range(CC):
                            nc.tensor.matmul(
                                pz,
                                lhsT=g_all[:, c1 * C + c2 * P : c1 * C + (c2 + 1) * P],
                                rhs=xT[:, c1 * 512 : (c1 + 1) * 512],
                                start=(c1 == 0),
                                stop=(c1 == CC - 1),
                            )
                        if c2 < 2:
                            nc.scalar.copy(z2[:, c2 * 512 : (c2 + 1) * 512], pz)
                        else:
                            nc.vector.tensor_copy(
                                z2[:, c2 * 512 : (c2 + 1) * 512], pz
                            )
                    self.est = [None, None]
                    self.vau = [None, None]
                    for b in range(2):
                        # fused V and ST stages: interleaving keeps the two
                        # PSUM groups and their evacuations overlapped
                        pv = pp.tile([P, 1024], F32, name="pv", tag="TV")
                        ps = pp.tile([P, 512], F32, name="pst", tag="PST")
                        for sc in range(2):
                            tlo = b * 256 + sc * P
                            tn = 256 if sc == 0 else P
                            dlo = sc * 256
                            for cc_ in range(CC):
                                xts = xT[:, cc_ * 512 + b * 256 + sc * P : cc_ * 512 + b * 256 + (sc + 1) * P]
                                nc.tensor.matmul(
                                    pv[:, sc * 512 : sc * 512 + H],
                                    lhsT=xts,
                                    rhs=wvT_all[:, cc_ * H : (cc_ + 1) * H],
                                    start=(cc_ == 0),
                                    stop=(cc_ == CC - 1),
                                    skip_group_check=True,
                                )
                                nc.tensor.matmul(
                                    ps[:, dlo : dlo + tn],
                                    lhsT=xts,
                                    rhs=z2[:, cc_ * 512 + tlo : cc_ * 512 + tlo + tn],
                                    start=(cc_ == 0),
                                    stop=(cc_ == CC - 1),
                                    skip_group_check=True,
                                )
                        # vau[p, sc*400 + h] ; ones at h in [384, 392)
                        vt = sb.tile([P, 800], BF16, name="vau", tag=f"vau{b}")
                        vdst = vt.rearrange("p (k c) -> p k c", k=2)[:, :, :H]
                        vsrc = pv.rearrange("p (k c) -> p k c", k=2)[:, :, :H]
                        if b == 0:
                            nc.vector.tensor_copy(vdst, vsrc)
                        else:
                            nc.scalar.copy(vdst, vsrc)
                        nc.gpsimd.memset(
                            vt.rearrange("p (k c) -> p k c", k=2)[:, :, H : H + 8],
                            1.0,
                        )
                        self.vau[b] = vt
                        # exp of both score blocks in one ACT op, bf16 out:
                        #   est cols [0:256] = sc0 (t in [0,256)),
                        #            [256:384] = sc1 (t in [128,256))
                        et = sb.tile([P, 384], BF16, name="est", tag=f"est{b}")
                        nc.scalar.activation(et, ps[:, :384], EXP)
                        # causal mask on the diagonal blocks (keep col >= p)
                        for cst in (0, 256):
                            nc.gpsimd.affine_select(
                                out=et[:, cst : cst + P],
                                in_=et[:, cst : cst + P],
                                compare_op=GE,
                                fill=0.0,
                                base=0,
                                channel_multiplier=-1,
                                pattern=[[1, P]],
                            )
                        self.est[b] = et

                def outs(self):
                    osb = sb.tile([P, 1536], F32, name="osb", tag="osb")
                    for b in range(2):
                        for tcc in range(2):
                            p_o = pp.tile([P, 512], F32, name="po", tag=f"PO{tcc}")
                            if tcc == 0:
                                nc.tensor.matmul(
                                    p_o[:, : H + 8],
                                    lhsT=self.est[b][:, 0:P],
                                    rhs=self.vau[b][:, 0:392],
                                    start=True,
                                    stop=True,
                                )
                            else:
                                nc.tensor.matmul(
                                    p_o[:, : H + 8],
                                    lhsT=self.est[b][:, P : 2 * P],
                                    rhs=self.vau[b][:, 0:392],
                                    start=True,
                                    stop=False,
                                )
                                nc.tensor.matmul(
                                    p_o[:, : H + 8],
                                    lhsT=self.est[b][:, 2 * P : 3 * P],
                                    rhs=self.vau[b][:, 400:792],
                                    start=False,
                                    stop=True,
                                )
                            rec = obp.tile([P, 1], F32, name="rec", tag="rec")
                            nc.vector.reciprocal(rec, p_o[:, H : H + 1])
                            dst = osb[:, tcc * 768 + b * 384 : tcc * 768 + (b + 1) * 384]
                            if tcc == 0:
                                nc.vector.tensor_scalar_mul(dst, p_o[:, :H], rec)
                            else:
                                nc.scalar.activation(
                                    dst, p_o[:, :H], COPY, scale=rec
                                )
                            nc.sync.dma_start(
                                out_d[self.b0 + b, tcc * P : (tcc + 1) * P, :],
                                dst,
                            )

            # pair 0's input DMA + transposes overlap the weight-load
            # and G/WvT setup below (they share no dependencies)
            pair0 = Pair(0)
            pair0.load_transpose()


            # Load weights: W[hc*128+p, c] -> wX_all[p, hc*384 + c], one DMA
            # each, rounding f32 -> f32r in the DMA so the G matmuls can run
            # in single-pass f32r mode
            wq_s, wk_s, wv_s = [], [], []
            for lst, srcd, nm in (
                (wq_s, wq_d, "wq"),
                (wk_s, wk_d, "wk"),
                (wv_s, wv_d, "wv"),
            ):
                w_all = cpool.tile([P, CC * C], mybir.dt.float32r, name=f"{nm}_all")
                nc.gpsimd.dma_start(
                    w_all.rearrange("p (hc c) -> p hc c", hc=CC),
                    srcd.rearrange("(hc p) c -> p hc c", hc=CC),
                )
                for hc in range(CC):
                    lst.append(w_all[:, hc * C : (hc + 1) * C])

            # G = (Wq.T @ Wk) * SCALE -> bf16  g_all[:, c1*384 + c2col]
            g_all = cpool.tile([P, CC * C], BF16, name="g_all")
            for c1 in range(CC):
                pg = pp.tile([P, 512], F32, name="pg", tag=("PST", "PO0", "PO1")[c1])
                for hc in range(CC):
                    nc.tensor.matmul(
                        pg[:, :C],
                        lhsT=wq_s[hc][:, c1 * P : (c1 + 1) * P],
                        rhs=wk_s[hc],
                        start=(hc == 0),
                        stop=(hc == CC - 1),
                    )
                nc.vector.tensor_scalar_mul(
                    g_all[:, c1 * C : (c1 + 1) * C], pg[:, :C], SCALE
                )

            # Wv -> bf16, then wvT_all[:, cc*384 + hc*128 + h'] bf16
            wvb = []
            for hc in range(CC):
                wb = cpool.tile([P, C], BF16, name=f"wvb{hc}")
                nc.vector.tensor_copy(wb, wv_s[hc])
                wvb.append(wb)
            pwt = pp.tile([P, 2048], BF16, name="pwt", tag="TV")
            for cc_ in range(CC):
                for hc in range(CC):
                    nc.tensor.transpose(
                        pwt[:, (cc_ * CC + hc) * P : (cc_ * CC + hc + 1) * P],
                        wvb[hc][:, cc_ * P : (cc_ + 1) * P],
                        ident,
                    )
            wvT_all = cpool.tile([P, CC * H], BF16, name="wvT_all")
            nc.vector.tensor_copy(wvT_all, pwt[:, : CC * H])

            prev = None
            for pr in range(npair):
                cur = pair0 if pr == 0 else Pair(pr)
                if pr > 0:
                    cur.load_transpose()
                if prev is not None:
                    # previous pair's output matmuls run while DVE drains
                    # this pair's transpose PSUM into xT
                    prev.outs()
                cur.z2_vst()
                prev = cur
            prev.outs()

    nc.compile()
    return nc


_NC_CACHE = {}


def _get_nc(nb: int):
    if nb not in _NC_CACHE:
        _NC_CACHE[nb] = build_bass(nb)
    return _NC_CACHE[nb]


def kernel(x: np.ndarray, Wk: np.ndarray, Wq: np.ndarray, Wv: np.ndarray, **_):
    x = np.ascontiguousarray(x, dtype=np.float32)
    Wk = np.ascontiguousarray(Wk, dtype=np.float32)
    Wq = np.ascontiguousarray(Wq, dtype=np.float32)
    Wv = np.ascontiguousarray(Wv, dtype=np.float32)
    nb = x.shape[0] // NCORES
    nc = _get_nc(nb)
    in_maps = [
        {"x": x[i * nb : (i + 1) * nb], "Wk": Wk, "Wq": Wq, "Wv": Wv}
        for i in range(NCORES)
    ]
    res = run_bass_kernel_spmd(nc, in_maps, core_ids=list(range(NCORES)))
    return np.concatenate([r["out"] for r in res.results], axis=0)


if __name__ == "__main__":
    rng = np.random.default_rng(0)
    x = rng.standard_normal((B, T, C), dtype=np.float32)
    s = 1.0 / np.sqrt(C)
    Wk = rng.standard_normal((H, C), dtype=np.float32) * s
    Wq = rng.standard_normal((H, C), dtype=np.float32) * s
    Wv = rng.standard_normal((H, C), dtype=np.float32) * s
    out = kernel(x=x, Wk=Wk, Wq=Wq, Wv=Wv)
    print(out.shape, out.dtype)


# revision 4
# speedup vs baseline: 1.0098x; 1.0098x over previous
"""Single-head causal self-attention on 8 Trainium2 NeuronCores.

Reference computation (per batch b):
    k = x @ Wk.T ; q = x @ Wq.T ; v = x @ Wv.T
    wei = softmax(mask(q @ k.T / sqrt(H)))
    out = wei @ v

Measured: ~142 us HW exec on 8 cores (baseline f32r kernel: ~300 us).

Design:
  - All matmul operands bf16 (PSUM accumulate fp32): bf16 stationary
    operands get Fast Weight Load and bf16 moving operands stream
    1 row/cycle.  Measured numerics: ~4e-3 rel err (tolerance 2e-2).
  - x is cast f32->bf16 INSIDE the input DMA (GpSimd SWDGE DMAs can
    convert dtypes) — no separate on-chip cast pass.
  - G = (Wq.T @ Wk) * scale precomputed once (q/k fusion halves the
    projection work).
  - Batches processed in pairs: transpose/z2 matmuls stream N=512.
  - Causal skips: the (s-chunk 1, t-chunk 0) score block is never
    computed; t-chunk 0 output only consumes s-chunk 0.
  - PSUM->SBUF evacuations are merged into single wide ops over
    multi-bank PSUM tiles and spread across DVE and ACT; pure-SBUF ops
    (masks, ones) go to GpSimd.  exp runs on ACT straight out of PSUM
    writing bf16; causal mask via affine_select on the two 128x128
    diagonal blocks only.
  - Softmax denominator via ones-columns appended to V; the final
    normalize is fused with the PSUM->SBUF copy (reciprocal on DVE,
    scaled copies split DVE/ACT).
  - PSUM plan (8 banks): TZ 3 banks (transposes as bf16, then z2),
    TV 2 banks (per-batch V), PST 1 bank (scores), PO0/PO1 1 bank each
    (output accumulators).
"""

import numpy as np

import concourse.bass as bass
import concourse.mybir as mybir
from concourse import bacc
import concourse.tile as tile
from concourse.bass_utils import run_bass_kernel_spmd
from concourse.masks import make_identity

B, T, C, H = 256, 256, 384, 384
NCORES = 8
NB = B // NCORES  # batches per core
P = 128
CC = C // P  # 3 chunks of the embedding dim
SCALE = float(H) ** -0.5
F32 = mybir.dt.float32
BF16 = mybir.dt.bfloat16

EXP = mybir.ActivationFunctionType.Exp
COPY = mybir.ActivationFunctionType.Copy
GE = mybir.AluOpType.is_ge


def build_bass(nb: int = NB):
    nc = bacc.Bacc(
        "TRN2",
        target_bir_lowering=False,
        debug=False,
        enable_asserts=False,
        num_devices=NCORES,
    )
    x_d = nc.dram_tensor("x", [nb, T, C], F32, kind="ExternalInput").ap()
    wk_d = nc.dram_tensor("Wk", [H, C], F32, kind="ExternalInput").ap()
    wq_d = nc.dram_tensor("Wq", [H, C], F32, kind="ExternalInput").ap()
    wv_d = nc.dram_tensor("Wv", [H, C], F32, kind="ExternalInput").ap()
    out_d = nc.dram_tensor("out", [nb, T, H], F32, kind="ExternalOutput").ap()

    assert nb % 2 == 0
    npair = nb // 2

    with tile.TileContext(nc) as tc:
        with (
            tc.tile_pool(name="const", bufs=1) as cpool,
            tc.tile_pool(name="sb", bufs=2) as sb,
            tc.tile_pool(name="ob", bufs=4) as obp,
            tc.tile_pool(name="pp", bufs=1, space="PSUM") as pp,
        ):
            ident = cpool.tile([P, P], BF16, name="ident")
            make_identity(nc, ident)

            class Pair:
                """Stage emitters for one batch pair; instances let pair
                k+1's transposes be issued before pair k's output matmuls
                (software pipelining across the in-order PE queue)."""

                def __init__(self, pr):
                    self.b0 = 2 * pr

                def load(self):
                    b0 = self.b0
                    # xb[p, u*768 + b*384 + c] = x[b0+b, u*128+p, c],
                    # cast f32->bf16 inside the DMA (GpSimd SWDGE)
                    xb = sb.tile([P, 1536], BF16, name="xb", tag="xb", bufs=3)
                    for u in range(2):
                        nc.gpsimd.dma_start(
                            xb[:, u * 768 : (u + 1) * 768],
                            x_d[b0 : b0 + 2, u * P : (u + 1) * P, :].rearrange(
                                "b t c -> t b c"
                            ),
                        )
                    self.xb = xb

                def transpose(self):
                    xb = self.xb
                    # transpose to xT[:, cc*512 + b*256 + u*128 + t']
                    xT = sb.tile([P, 1536], BF16, name="xT", tag="xT", bufs=3)
                    for cc_ in range(CC):
                        ptx = pp.tile([P, 1024], BF16, name="ptx", tag=f"T{cc_}")
                        for b in range(2):
                            for u in range(2):
                                nc.tensor.transpose(
                                    ptx[:, b * 256 + u * P : b * 256 + (u + 1) * P],
                                    xb[:, u * 768 + b * 384 + cc_ * P : u * 768 + b * 384 + (cc_ + 1) * P],
                                    ident,
                                )
                        nc.vector.tensor_copy(
                            xT[:, cc_ * 512 : (cc_ + 1) * 512], ptx[:, :512]
                        )
                    self.xT = xT

                def z2_vst(self):
                    xT = self.xT
                    # z2[c2] = (G.T @ xT): z2[:, c2*512 + b*256 + u*128 + t']
                    z2 = sb.tile([P, 1536], BF16, name="z2", tag="z2")
                    for c2 in range(CC):
                        pz = pp.tile([P, 512], F32, name="pz", tag=f"T{c2}")
                        for c1 in range(CC):
                            nc.tensor.matmul(
                                pz,
                                lhsT=g_all[:, c1 * C + c2 * P : c1 * C + (c2 + 1) * P],
                                rhs=xT[:, c1 * 512 : (c1 + 1) * 512],
                                start=(c1 == 0),
                                stop=(c1 == CC - 1),
                            )
                        if c2 < 2:
                            nc.scalar.copy(z2[:, c2 * 512 : (c2 + 1) * 512], pz)
                        else:
                            nc.vector.tensor_copy(
                                z2[:, c2 * 512 : (c2 + 1) * 512], pz
                            )
                    self.est = [None, None]
                    self.vau = [None, None]
                    for b in range(2):
                        # fused V and ST stages: interleaving keeps the two
                        # PSUM groups and their evacuations overlapped
                        pv = pp.tile([P, 1024], F32, name="pv", tag="TV")
                        ps = pp.tile([P, 512], F32, name="pst", tag="PST")
                        for sc in range(2):
                            tlo = b * 256 + sc * P
                            tn = 256 if sc == 0 else P
                            dlo = sc * 256
                            for cc_ in range(CC):
                                xts = xT[:, cc_ * 512 + b * 256 + sc * P : cc_ * 512 + b * 256 + (sc + 1) * P]
                                nc.tensor.matmul(
                                    pv[:, sc * 512 : sc * 512 + H],
                                    lhsT=xts,
                                    rhs=wvT_all[:, cc_ * H : (cc_ + 1) * H],
                                    start=(cc_ == 0),
                                    stop=(cc_ == CC - 1),
                                    skip_group_check=True,
                                )
                                nc.tensor.matmul(
                                    ps[:, dlo : dlo + tn],
                                    lhsT=xts,
                                    rhs=z2[:, cc_ * 512 + tlo : cc_ * 512 + tlo + tn],
                                    start=(cc_ == 0),
                                    stop=(cc_ == CC - 1),
                                    skip_group_check=True,
                                )
                        # vau[p, sc*400 + h] ; ones at h in [384, 392)
                        vt = sb.tile([P, 800], BF16, name="vau", tag=f"vau{b}")
                        vdst = vt.rearrange("p (k c) -> p k c", k=2)[:, :, :H]
                        vsrc = pv.rearrange("p (k c) -> p k c", k=2)[:, :, :H]
                        if b == 0:
                            nc.vector.tensor_copy(vdst, vsrc)
                        else:
                            nc.scalar.copy(vdst, vsrc)
                        nc.gpsimd.memset(
                            vt.rearrange("p (k c) -> p k c", k=2)[:, :, H : H + 8],
                            1.0,
                        )
                        self.vau[b] = vt
                        # exp of both score blocks in one ACT op, bf16 out:
                        #   est cols [0:256] = sc0 (t in [0,256)),
                        #            [256:384] = sc1 (t in [128,256))
                        et = sb.tile([P, 384], BF16, name="est", tag=f"est{b}")
                        nc.scalar.activation(et, ps[:, :384], EXP)
                        # causal mask on the diagonal blocks (keep col >= p)
                        for cst in (0, 256):
                            nc.gpsimd.affine_select(
                                out=et[:, cst : cst + P],
                                in_=et[:, cst : cst + P],
                                compare_op=GE,
                                fill=0.0,
                                base=0,
                                channel_multiplier=-1,
                                pattern=[[1, P]],
                            )
                        self.est[b] = et

                def outs(self):
                    osb = sb.tile([P, 1536], F32, name="osb", tag="osb")
                    for b in range(2):
                        for tcc in range(2):
                            p_o = pp.tile([P, 512], F32, name="po", tag=f"PO{tcc}")
                            if tcc == 0:
                                nc.tensor.matmul(
                                    p_o[:, : H + 8],
                                    lhsT=self.est[b][:, 0:P],
                                    rhs=self.vau[b][:, 0:392],
                                    start=True,
                                    stop=True,
                                )
                            else:
                                nc.tensor.matmul(
                                    p_o[:, : H + 8],
                                    lhsT=self.est[b][:, P : 2 * P],
                                    rhs=self.vau[b][:, 0:392],
                                    start=True,
                                    stop=False,
                                )
                                nc.tensor.matmul(
                                    p_o[:, : H + 8],
                                    lhsT=self.est[b][:, 2 * P : 3 * P],
                                    rhs=self.vau[b][:, 400:792],
                                    start=False,
                                    stop=True,
                                )
                            rec = obp.tile([P, 1], F32, name="rec", tag="rec")
                            nc.vector.reciprocal(rec, p_o[:, H : H + 1])
                            dst = osb[:, tcc * 768 + b * 384 : tcc * 768 + (b + 1) * 384]
                            if tcc == 0:
                                nc.vector.tensor_scalar_mul(dst, p_o[:, :H], rec)
                            else:
                                nc.scalar.activation(
                                    dst, p_o[:, :H], COPY, scale=rec
                                )
                            nc.sync.dma_start(
                                out_d[self.b0 + b, tcc * P : (tcc + 1) * P, :],
                                dst,
                            )

            # pair 0's input DMA + transposes overlap the weight-load
            # and G/WvT setup below (they share no dependencies)
            pair0 = Pair(0)
            pair0.load()
            pair0.transpose()


            # Load weights: W[hc*128+p, c] -> wX_all[p, hc*384 + c], one DMA
            # each, rounding f32 -> f32r in the DMA so the G matmuls can run
            # in single-pass f32r mode
            wq_s, wk_s, wv_s = [], [], []
            for lst, srcd, nm in (
                (wq_s, wq_d, "wq"),
                (wk_s, wk_d, "wk"),
                (wv_s, wv_d, "wv"),
            ):
                w_all = cpool.tile([P, CC * C], mybir.dt.float32r, name=f"{nm}_all")
                nc.gpsimd.dma_start(
                    w_all.rearrange("p (hc c) -> p hc c", hc=CC),
                    srcd.rearrange("(hc p) c -> p hc c", hc=CC),
                )
                for hc in range(CC):
                    lst.append(w_all[:, hc * C : (hc + 1) * C])

            # G = (Wq.T @ Wk) * SCALE -> bf16  g_all[:, c1*384 + c2col]
            g_all = cpool.tile([P, CC * C], BF16, name="g_all")
            for c1 in range(CC):
                pg = pp.tile([P, 512], F32, name="pg", tag=("PST", "PO0", "PO1")[c1])
                for hc in range(CC):
                    nc.tensor.matmul(
                        pg[:, :C],
                        lhsT=wq_s[hc][:, c1 * P : (c1 + 1) * P],
                        rhs=wk_s[hc],
                        start=(hc == 0),
                        stop=(hc == CC - 1),
                    )
                nc.vector.tensor_scalar_mul(
                    g_all[:, c1 * C : (c1 + 1) * C], pg[:, :C], SCALE
                )

            # Wv -> bf16, then wvT_all[:, cc*384 + hc*128 + h'] bf16
            wvb = []
            for hc in range(CC):
                wb = cpool.tile([P, C], BF16, name=f"wvb{hc}")
                nc.vector.tensor_copy(wb, wv_s[hc])
                wvb.append(wb)
            pwt = pp.tile([P, 2048], BF16, name="pwt", tag="TV")
            for cc_ in range(CC):
                for hc in range(CC):
                    nc.tensor.transpose(
                        pwt[:, (cc_ * CC + hc) * P : (cc_ * CC + hc + 1) * P],
                        wvb[hc][:, cc_ * P : (cc_ + 1) * P],
                        ident,
                    )
            wvT_all = cpool.tile([P, CC * H], BF16, name="wvT_all")
            nc.vector.tensor_copy(wvT_all, pwt[:, : CC * H])

            pairs = [pair0] + [Pair(pr) for pr in range(1, npair)]
            if npair > 1:
                pairs[1].load()  # prefetch
            prev = None
            for pr in range(npair):
                cur = pairs[pr]
                if pr > 0:
                    cur.transpose()
                if pr + 2 < npair:
                    # issue pair k+2's input DMA a full pair early so the
                    # transfer fully overlaps compute
                    pairs[pr + 2].load()
                if prev is not None:
                    # previous pair's output matmuls run while DVE drains
                    # this pair's transpose PSUM into xT
                    prev.outs()
                cur.z2_vst()
                prev = cur
            prev.outs()

    nc.compile()
    return nc


_NC_CACHE = {}


def _get_nc(nb: int):
    if nb not in _NC_CACHE:
        _NC_CACHE[nb] = build_bass(nb)
    return _NC_CACHE[nb]


def kernel(x: np.ndarray, Wk: np.ndarray, Wq: np.ndarray, Wv: np.ndarray, **_):
    x = np.ascontiguousarray(x, dtype=np.float32)
    Wk = np.ascontiguousarray(Wk, dtype=np.float32)
    Wq = np.ascontiguousarray(Wq, dtype=np.float32)
    Wv = np.ascontiguousarray(Wv, dtype=np.float32)
    nb = x.shape[0] // NCORES
    nc = _get_nc(nb)
    in_maps = [
        {"x": x[i * nb : (i + 1) * nb], "Wk": Wk, "Wq": Wq, "Wv": Wv}
        for i in range(NCORES)
    ]
    res = run_bass_kernel_spmd(nc, in_maps, core_ids=list(range(NCORES)))
    return np.concatenate([r["out"] for r in res.results], axis=0)


if __name__ == "__main__":
    rng = np.random.default_rng(0)
    x = rng.standard_normal((B, T, C), dtype=np.float32)
    s = 1.0 / np.sqrt(C)
    Wk = rng.standard_normal((H, C), dtype=np.float32) * s
    Wq = rng.standard_normal((H, C), dtype=np.float32) * s
    Wv = rng.standard_normal((H, C), dtype=np.float32) * s
    out = kernel(x=x, Wk=Wk, Wq=Wq, Wv=Wv)
    print(out.shape, out.dtype)


# revision 5
# speedup vs baseline: 1.0217x; 1.0117x over previous
"""Single-head causal self-attention on 8 Trainium2 NeuronCores.

Reference computation (per batch b):
    k = x @ Wk.T ; q = x @ Wq.T ; v = x @ Wv.T
    wei = softmax(mask(q @ k.T / sqrt(H)))
    out = wei @ v

Measured: ~141.5 us HW exec on 8 cores (baseline f32r kernel: ~300 us).

Design:
  - All matmul operands bf16 (PSUM accumulate fp32): bf16 stationary
    operands get Fast Weight Load and bf16 moving operands stream
    1 row/cycle.  Measured numerics: ~4e-3 rel err (tolerance 2e-2).
  - x is cast f32->bf16 INSIDE the input DMA (GpSimd SWDGE DMAs can
    convert dtypes) — no separate on-chip cast pass.
  - G = (Wq.T @ Wk) * scale precomputed once (q/k fusion halves the
    projection work).
  - Batches processed in pairs: transpose/z2 matmuls stream N=512.
  - Causal skips: the (s-chunk 1, t-chunk 0) score block is never
    computed; t-chunk 0 output only consumes s-chunk 0.
  - PSUM->SBUF evacuations are merged into single wide ops over
    multi-bank PSUM tiles and spread across DVE and ACT; pure-SBUF ops
    (masks, ones) go to GpSimd.  exp runs on ACT straight out of PSUM
    writing bf16; causal mask via affine_select on the two 128x128
    diagonal blocks only.
  - Softmax denominator via ones-columns appended to V; the final
    normalize is fused with the PSUM->SBUF copy (reciprocal on DVE,
    scaled copies split DVE/ACT).
  - PSUM plan (8 banks): TZ 3 banks (transposes as bf16, then z2),
    TV 2 banks (per-batch V), PST 1 bank (scores), PO0/PO1 1 bank each
    (output accumulators).
"""

import numpy as np

import concourse.bass as bass
import concourse.mybir as mybir
from concourse import bacc
import concourse.tile as tile
from concourse.bass_utils import run_bass_kernel_spmd
from concourse.masks import make_identity

B, T, C, H = 256, 256, 384, 384
NCORES = 8
NB = B // NCORES  # batches per core
P = 128
CC = C // P  # 3 chunks of the embedding dim
SCALE = float(H) ** -0.5
F32 = mybir.dt.float32
BF16 = mybir.dt.bfloat16

EXP = mybir.ActivationFunctionType.Exp
COPY = mybir.ActivationFunctionType.Copy
GE = mybir.AluOpType.is_ge


def build_bass(nb: int = NB):
    nc = bacc.Bacc(
        "TRN2",
        target_bir_lowering=False,
        debug=False,
        enable_asserts=False,
        num_devices=NCORES,
    )
    x_d = nc.dram_tensor("x", [nb, T, C], F32, kind="ExternalInput").ap()
    wk_d = nc.dram_tensor("Wk", [H, C], F32, kind="ExternalInput").ap()
    wq_d = nc.dram_tensor("Wq", [H, C], F32, kind="ExternalInput").ap()
    wv_d = nc.dram_tensor("Wv", [H, C], F32, kind="ExternalInput").ap()
    out_d = nc.dram_tensor("out", [nb, T, H], F32, kind="ExternalOutput").ap()

    assert nb % 2 == 0
    npair = nb // 2

    with tile.TileContext(nc) as tc:
        with (
            tc.tile_pool(name="const", bufs=1) as cpool,
            tc.tile_pool(name="sb", bufs=2) as sb,
            tc.tile_pool(name="ob", bufs=4) as obp,
            tc.tile_pool(name="pp", bufs=1, space="PSUM") as pp,
        ):
            ident = cpool.tile([P, P], BF16, name="ident")
            make_identity(nc, ident)

            class Pair:
                """Stage emitters for one batch pair; instances let pair
                k+1's transposes be issued before pair k's output matmuls
                (software pipelining across the in-order PE queue)."""

                def __init__(self, pr):
                    self.b0 = 2 * pr

                def load(self):
                    b0 = self.b0
                    # xb[p, u*768 + b*384 + c] = x[b0+b, u*128+p, c],
                    # cast f32->bf16 inside the DMA (GpSimd SWDGE)
                    xb = sb.tile([P, 1536], BF16, name="xb", tag="xb", bufs=3)
                    for u in range(2):
                        nc.gpsimd.dma_start(
                            xb[:, u * 768 : (u + 1) * 768],
                            x_d[b0 : b0 + 2, u * P : (u + 1) * P, :].rearrange(
                                "b t c -> t b c"
                            ),
                        )
                    self.xb = xb

                def transpose(self):
                    xb = self.xb
                    # transpose to xT[:, cc*512 + b*256 + u*128 + t']
                    xT = sb.tile([P, 1536], BF16, name="xT", tag="xT", bufs=3)
                    for cc_ in range(CC):
                        ptx = pp.tile([P, 1024], BF16, name="ptx", tag=f"T{cc_}")
                        for b in range(2):
                            for u in range(2):
                                nc.tensor.transpose(
                                    ptx[:, b * 256 + u * P : b * 256 + (u + 1) * P],
                                    xb[:, u * 768 + b * 384 + cc_ * P : u * 768 + b * 384 + (cc_ + 1) * P],
                                    ident,
                                )
                        nc.vector.tensor_copy(
                            xT[:, cc_ * 512 : (cc_ + 1) * 512], ptx[:, :512]
                        )
                    self.xT = xT

                def z2_mms(self):
                    xT = self.xT
                    # z2[c2] = (G.T @ xT): z2[:, c2*512 + b*256 + u*128 + t']
                    z2 = sb.tile([P, 1536], BF16, name="z2", tag="z2")
                    for c2 in range(CC):
                        pz = pp.tile([P, 512], F32, name="pz", tag=f"T{c2}")
                        for c1 in range(CC):
                            nc.tensor.matmul(
                                pz,
                                lhsT=g_all[:, c1 * C + c2 * P : c1 * C + (c2 + 1) * P],
                                rhs=xT[:, c1 * 512 : (c1 + 1) * 512],
                                start=(c1 == 0),
                                stop=(c1 == CC - 1),
                            )
                        if c2 < 2:
                            nc.scalar.copy(z2[:, c2 * 512 : (c2 + 1) * 512], pz)
                        else:
                            nc.vector.tensor_copy(
                                z2[:, c2 * 512 : (c2 + 1) * 512], pz
                            )
                    self.z2 = z2

                def vst(self):
                    xT = self.xT
                    z2 = self.z2
                    self.est = [None, None]
                    self.vau = [None, None]
                    for b in range(2):
                        # fused V and ST stages: interleaving keeps the two
                        # PSUM groups and their evacuations overlapped
                        pv = pp.tile([P, 1024], F32, name="pv", tag="TV")
                        ps = pp.tile([P, 512], F32, name="pst", tag="PST")
                        for sc in range(2):
                            tlo = b * 256 + sc * P
                            tn = 256 if sc == 0 else P
                            dlo = sc * 256
                            for cc_ in range(CC):
                                xts = xT[:, cc_ * 512 + b * 256 + sc * P : cc_ * 512 + b * 256 + (sc + 1) * P]
                                nc.tensor.matmul(
                                    pv[:, sc * 512 : sc * 512 + H],
                                    lhsT=xts,
                                    rhs=wvT_all[:, cc_ * H : (cc_ + 1) * H],
                                    start=(cc_ == 0),
                                    stop=(cc_ == CC - 1),
                                    skip_group_check=True,
                                )
                                nc.tensor.matmul(
                                    ps[:, dlo : dlo + tn],
                                    lhsT=xts,
                                    rhs=z2[:, cc_ * 512 + tlo : cc_ * 512 + tlo + tn],
                                    start=(cc_ == 0),
                                    stop=(cc_ == CC - 1),
                                    skip_group_check=True,
                                )
                        # vau[p, sc*400 + h] ; ones at h in [384, 392)
                        vt = sb.tile([P, 800], BF16, name="vau", tag=f"vau{b}")
                        vdst = vt.rearrange("p (k c) -> p k c", k=2)[:, :, :H]
                        vsrc = pv.rearrange("p (k c) -> p k c", k=2)[:, :, :H]
                        if b == 0:
                            nc.vector.tensor_copy(vdst, vsrc)
                        else:
                            nc.scalar.copy(vdst, vsrc)
                        nc.gpsimd.memset(
                            vt.rearrange("p (k c) -> p k c", k=2)[:, :, H : H + 8],
                            1.0,
                        )
                        self.vau[b] = vt
                        # exp of both score blocks in one ACT op, bf16 out:
                        #   est cols [0:256] = sc0 (t in [0,256)),
                        #            [256:384] = sc1 (t in [128,256))
                        et = sb.tile([P, 384], BF16, name="est", tag=f"est{b}")
                        nc.scalar.activation(et, ps[:, :384], EXP)
                        # causal mask on the diagonal blocks (keep col >= p)
                        for cst in (0, 256):
                            nc.gpsimd.affine_select(
                                out=et[:, cst : cst + P],
                                in_=et[:, cst : cst + P],
                                compare_op=GE,
                                fill=0.0,
                                base=0,
                                channel_multiplier=-1,
                                pattern=[[1, P]],
                            )
                        self.est[b] = et

                def out_mms(self):
                    self.osb = sb.tile([P, 1536], F32, name="osb", tag="osb")
                    self.po = []
                    for b in range(2):
                        for tcc in range(2):
                            p_o = pp.tile([P, 512], F32, name="po", tag=f"PO{tcc}")
                            if tcc == 0:
                                nc.tensor.matmul(
                                    p_o[:, : H + 8],
                                    lhsT=self.est[b][:, 0:P],
                                    rhs=self.vau[b][:, 0:392],
                                    start=True,
                                    stop=True,
                                )
                            else:
                                nc.tensor.matmul(
                                    p_o[:, : H + 8],
                                    lhsT=self.est[b][:, P : 2 * P],
                                    rhs=self.vau[b][:, 0:392],
                                    start=True,
                                    stop=False,
                                )
                                nc.tensor.matmul(
                                    p_o[:, : H + 8],
                                    lhsT=self.est[b][:, 2 * P : 3 * P],
                                    rhs=self.vau[b][:, 400:792],
                                    start=False,
                                    stop=True,
                                )
                            self.po.append(p_o)

                def out_norms(self):
                    osb = self.osb
                    for b in range(2):
                        for tcc in range(2):
                            p_o = self.po[b * 2 + tcc]
                            rec = obp.tile([P, 1], F32, name="rec", tag="rec")
                            nc.vector.reciprocal(rec, p_o[:, H : H + 1])
                            dst = osb[:, tcc * 768 + b * 384 : tcc * 768 + (b + 1) * 384]
                            if tcc == 0:
                                nc.vector.tensor_scalar_mul(dst, p_o[:, :H], rec)
                            else:
                                nc.scalar.activation(
                                    dst, p_o[:, :H], COPY, scale=rec
                                )
                            nc.sync.dma_start(
                                out_d[self.b0 + b, tcc * P : (tcc + 1) * P, :],
                                dst,
                            )

            # pair 0's input DMA + transposes overlap the weight-load
            # and G/WvT setup below (they share no dependencies)
            pair0 = Pair(0)
            pair0.load()
            pair0.transpose()


            # Load weights: W[hc*128+p, c] -> wX_all[p, hc*384 + c], one DMA
            # each, rounding f32 -> bf16 in the DMA so the G matmuls run at
            # bf16 speed with fast weight load (G error contribution is well
            # inside the tolerance; verified in CoreSim)
            wq_s, wk_s, wv_s = [], [], []
            for lst, srcd, nm in (
                (wq_s, wq_d, "wq"),
                (wk_s, wk_d, "wk"),
                (wv_s, wv_d, "wv"),
            ):
                w_all = cpool.tile([P, CC * C], BF16, name=f"{nm}_all")
                nc.gpsimd.dma_start(
                    w_all.rearrange("p (hc c) -> p hc c", hc=CC),
                    srcd.rearrange("(hc p) c -> p hc c", hc=CC),
                )
                for hc in range(CC):
                    lst.append(w_all[:, hc * C : (hc + 1) * C])

            # G = (Wq.T @ Wk) * SCALE -> bf16  g_all[:, c1*384 + c2col]
            g_all = cpool.tile([P, CC * C], BF16, name="g_all")
            for c1 in range(CC):
                pg = pp.tile([P, 512], F32, name="pg", tag=("PST", "PO0", "PO1")[c1])
                for hc in range(CC):
                    nc.tensor.matmul(
                        pg[:, :C],
                        lhsT=wq_s[hc][:, c1 * P : (c1 + 1) * P],
                        rhs=wk_s[hc],
                        start=(hc == 0),
                        stop=(hc == CC - 1),
                    )
                nc.vector.tensor_scalar_mul(
                    g_all[:, c1 * C : (c1 + 1) * C], pg[:, :C], SCALE
                )

            # wvT_all[:, cc*384 + hc*128 + h'] bf16 (wv_s already bf16)
            pwt = pp.tile([P, 2048], BF16, name="pwt", tag="TV")
            for cc_ in range(CC):
                for hc in range(CC):
                    nc.tensor.transpose(
                        pwt[:, (cc_ * CC + hc) * P : (cc_ * CC + hc + 1) * P],
                        wv_s[hc][:, cc_ * P : (cc_ + 1) * P],
                        ident,
                    )
            wvT_all = cpool.tile([P, CC * H], BF16, name="wvT_all")
            nc.vector.tensor_copy(wvT_all, pwt[:, : CC * H])

            pairs = [pair0] + [Pair(pr) for pr in range(1, npair)]
            if npair > 1:
                pairs[1].load()  # prefetch
            prev = None
            for pr in range(npair):
                cur = pairs[pr]
                if pr > 0:
                    cur.transpose()
                if pr + 2 < npair:
                    # issue pair k+2's input DMA a full pair early so the
                    # transfer fully overlaps compute
                    pairs[pr + 2].load()
                if prev is not None:
                    # previous pair's output matmuls run while DVE drains
                    # this pair's transpose PSUM into xT
                    prev.out_mms()
                    prev.out_norms()
                cur.z2_mms()
                cur.vst()
                prev = cur
            prev.out_mms()
            prev.out_norms()

    nc.compile()
    return nc


_NC_CACHE = {}


def _get_nc(nb: int):
    if nb not in _NC_CACHE:
        _NC_CACHE[nb] = build_bass(nb)
    return _NC_CACHE[nb]


def kernel(x: np.ndarray, Wk: np.ndarray, Wq: np.ndarray, Wv: np.ndarray, **_):
    x = np.ascontiguousarray(x, dtype=np.float32)
    Wk = np.ascontiguousarray(Wk, dtype=np.float32)
    Wq = np.ascontiguousarray(Wq, dtype=np.float32)
    Wv = np.ascontiguousarray(Wv, dtype=np.float32)
    nb = x.shape[0] // NCORES
    nc = _get_nc(nb)
    in_maps = [
        {"x": x[i * nb : (i + 1) * nb], "Wk": Wk, "Wq": Wq, "Wv": Wv}
        for i in range(NCORES)
    ]
    res = run_bass_kernel_spmd(nc, in_maps, core_ids=list(range(NCORES)))
    return np.concatenate([r["out"] for r in res.results], axis=0)


if __name__ == "__main__":
    rng = np.random.default_rng(0)
    x = rng.standard_normal((B, T, C), dtype=np.float32)
    s = 1.0 / np.sqrt(C)
    Wk = rng.standard_normal((H, C), dtype=np.float32) * s
    Wq = rng.standard_normal((H, C), dtype=np.float32) * s
    Wv = rng.standard_normal((H, C), dtype=np.float32) * s
    out = kernel(x=x, Wk=Wk, Wq=Wq, Wv=Wv)
    print(out.shape, out.dtype)


# revision 6
# speedup vs baseline: 1.0230x; 1.0013x over previous
"""Single-head causal self-attention on 8 Trainium2 NeuronCores.

Reference computation (per batch b):
    k = x @ Wk.T ; q = x @ Wq.T ; v = x @ Wv.T
    wei = softmax(mask(q @ k.T / sqrt(H)))
    out = wei @ v

Measured: ~140 us HW exec on 8 cores (baseline f32r kernel: ~300 us).

Design:
  - All matmul operands bf16 (PSUM accumulate fp32): bf16 stationary
    operands get Fast Weight Load and bf16 moving operands stream
    1 row/cycle.  Measured numerics: ~4e-3 rel err (tolerance 2e-2).
  - x is cast f32->bf16 INSIDE the input DMA (GpSimd SWDGE DMAs can
    convert dtypes) — no separate on-chip cast pass.
  - G = (Wq.T @ Wk) * scale precomputed once (q/k fusion halves the
    projection work).
  - Batches processed in pairs: transpose/z2 matmuls stream N=512.
  - Causal skips: the (s-chunk 1, t-chunk 0) score block is never
    computed; t-chunk 0 output only consumes s-chunk 0.
  - PSUM->SBUF evacuations are merged into single wide ops over
    multi-bank PSUM tiles and spread across DVE and ACT; pure-SBUF ops
    (masks, ones) go to GpSimd.  exp runs on ACT straight out of PSUM
    writing bf16; causal mask via affine_select on the two 128x128
    diagonal blocks only.
  - Softmax denominator via ones-columns appended to V; the final
    normalize is fused with the PSUM->SBUF copy (reciprocal on DVE,
    scaled copies split DVE/ACT).
  - PSUM plan (8 banks): TZ 3 banks (transposes as bf16, then z2),
    TV 2 banks (per-batch V), PST 1 bank (scores), PO0/PO1 1 bank each
    (output accumulators).
"""

import numpy as np

import concourse.bass as bass
import concourse.mybir as mybir
from concourse import bacc
import concourse.tile as tile
from concourse.bass_utils import run_bass_kernel_spmd
from concourse.masks import make_identity

B, T, C, H = 256, 256, 384, 384
NCORES = 8
NB = B // NCORES  # batches per core
P = 128
CC = C // P  # 3 chunks of the embedding dim
SCALE = float(H) ** -0.5
F32 = mybir.dt.float32
BF16 = mybir.dt.bfloat16

EXP = mybir.ActivationFunctionType.Exp
COPY = mybir.ActivationFunctionType.Copy
GE = mybir.AluOpType.is_ge


def build_bass(nb: int = NB):
    nc = bacc.Bacc(
        "TRN2",
        target_bir_lowering=False,
        debug=False,
        enable_asserts=False,
        num_devices=NCORES,
    )
    x_d = nc.dram_tensor("x", [nb, T, C], F32, kind="ExternalInput").ap()
    wk_d = nc.dram_tensor("Wk", [H, C], F32, kind="ExternalInput").ap()
    wq_d = nc.dram_tensor("Wq", [H, C], F32, kind="ExternalInput").ap()
    wv_d = nc.dram_tensor("Wv", [H, C], F32, kind="ExternalInput").ap()
    out_d = nc.dram_tensor("out", [nb, T, H], F32, kind="ExternalOutput").ap()

    assert nb % 2 == 0
    npair = nb // 2

    with tile.TileContext(nc) as tc:
        with (
            tc.tile_pool(name="const", bufs=1) as cpool,
            tc.tile_pool(name="sb", bufs=2) as sb,
            tc.tile_pool(name="ob", bufs=4) as obp,
            tc.tile_pool(name="pp", bufs=1, space="PSUM") as pp,
        ):
            ident = cpool.tile([P, P], BF16, name="ident")
            make_identity(nc, ident)

            class Pair:
                """Stage emitters for one batch pair; instances let pair
                k+1's transposes be issued before pair k's output matmuls
                (software pipelining across the in-order PE queue)."""

                def __init__(self, pr):
                    self.b0 = 2 * pr

                def load(self):
                    b0 = self.b0
                    # xb[p, b*768 + u*384 + c] = x[b0+b, u*128+p, c] in ONE
                    # cast-DMA: the (b,u) block pair has uniform stride in
                    # DRAM so it merges into a single AP dim
                    xb = sb.tile([P, 1536], BF16, name="xb", tag="xb", bufs=3)
                    nc.gpsimd.dma_start(
                        xb,
                        x_d[b0 : b0 + 2, :, :].rearrange(
                            "b (u t) c -> t (b u) c", u=2
                        ),
                    )
                    self.xb = xb

                def transpose(self):
                    xb = self.xb
                    # transpose to xT[:, cc*512 + b*256 + u*128 + t']
                    xT = sb.tile([P, 1536], BF16, name="xT", tag="xT", bufs=3)
                    for cc_ in range(CC):
                        ptx = pp.tile([P, 1024], BF16, name="ptx", tag=f"T{cc_}")
                        for b in range(2):
                            for u in range(2):
                                nc.tensor.transpose(
                                    ptx[:, b * 256 + u * P : b * 256 + (u + 1) * P],
                                    xb[:, b * 768 + u * 384 + cc_ * P : b * 768 + u * 384 + (cc_ + 1) * P],
                                    ident,
                                )
                        nc.vector.tensor_copy(
                            xT[:, cc_ * 512 : (cc_ + 1) * 512], ptx[:, :512]
                        )
                    self.xT = xT

                def z2_mms(self):
                    xT = self.xT
                    # z2[c2] = (G.T @ xT): z2[:, c2*512 + b*256 + u*128 + t']
                    z2 = sb.tile([P, 1536], BF16, name="z2", tag="z2")
                    for c2 in range(CC):
                        pz = pp.tile([P, 512], F32, name="pz", tag=f"T{c2}")
                        for c1 in range(CC):
                            nc.tensor.matmul(
                                pz,
                                lhsT=g_all[:, c1 * C + c2 * P : c1 * C + (c2 + 1) * P],
                                rhs=xT[:, c1 * 512 : (c1 + 1) * 512],
                                start=(c1 == 0),
                                stop=(c1 == CC - 1),
                            )
                        if c2 < 2:
                            nc.scalar.copy(z2[:, c2 * 512 : (c2 + 1) * 512], pz)
                        else:
                            nc.vector.tensor_copy(
                                z2[:, c2 * 512 : (c2 + 1) * 512], pz
                            )
                    self.z2 = z2

                def vst(self):
                    xT = self.xT
                    z2 = self.z2
                    self.est = [None, None]
                    self.vau = [None, None]
                    for b in range(2):
                        # fused V and ST stages: interleaving keeps the two
                        # PSUM groups and their evacuations overlapped
                        pv = pp.tile([P, 1024], F32, name="pv", tag="TV")
                        ps = pp.tile([P, 512], F32, name="pst", tag="PST")
                        for sc in range(2):
                            tlo = b * 256 + sc * P
                            tn = 256 if sc == 0 else P
                            dlo = sc * 256
                            for cc_ in range(CC):
                                xts = xT[:, cc_ * 512 + b * 256 + sc * P : cc_ * 512 + b * 256 + (sc + 1) * P]
                                nc.tensor.matmul(
                                    pv[:, sc * 512 : sc * 512 + H],
                                    lhsT=xts,
                                    rhs=wvT_all[:, cc_ * H : (cc_ + 1) * H],
                                    start=(cc_ == 0),
                                    stop=(cc_ == CC - 1),
                                    skip_group_check=True,
                                )
                                nc.tensor.matmul(
                                    ps[:, dlo : dlo + tn],
                                    lhsT=xts,
                                    rhs=z2[:, cc_ * 512 + tlo : cc_ * 512 + tlo + tn],
                                    start=(cc_ == 0),
                                    stop=(cc_ == CC - 1),
                                    skip_group_check=True,
                                )
                        # vau[p, sc*400 + h] ; ones at h in [384, 392)
                        vt = sb.tile([P, 800], BF16, name="vau", tag=f"vau{b}")
                        vdst = vt.rearrange("p (k c) -> p k c", k=2)[:, :, :H]
                        vsrc = pv.rearrange("p (k c) -> p k c", k=2)[:, :, :H]
                        if b == 0:
                            nc.vector.tensor_copy(vdst, vsrc)
                        else:
                            nc.scalar.copy(vdst, vsrc)
                        nc.gpsimd.memset(
                            vt.rearrange("p (k c) -> p k c", k=2)[:, :, H : H + 8],
                            1.0,
                        )
                        self.vau[b] = vt
                        # exp of both score blocks in one ACT op, bf16 out:
                        #   est cols [0:256] = sc0 (t in [0,256)),
                        #            [256:384] = sc1 (t in [128,256))
                        et = sb.tile([P, 384], BF16, name="est", tag=f"est{b}")
                        nc.scalar.activation(et, ps[:, :384], EXP)
                        # causal mask on the diagonal blocks (keep col >= p)
                        for cst in (0, 256):
                            nc.gpsimd.affine_select(
                                out=et[:, cst : cst + P],
                                in_=et[:, cst : cst + P],
                                compare_op=GE,
                                fill=0.0,
                                base=0,
                                channel_multiplier=-1,
                                pattern=[[1, P]],
                            )
                        self.est[b] = et

                def out_mms(self):
                    self.osb = sb.tile([P, 1536], F32, name="osb", tag="osb")
                    self.po = []
                    for b in range(2):
                        for tcc in range(2):
                            p_o = pp.tile([P, 512], F32, name="po", tag=f"PO{tcc}")
                            if tcc == 0:
                                nc.tensor.matmul(
                                    p_o[:, : H + 8],
                                    lhsT=self.est[b][:, 0:P],
                                    rhs=self.vau[b][:, 0:392],
                                    start=True,
                                    stop=True,
                                )
                            else:
                                nc.tensor.matmul(
                                    p_o[:, : H + 8],
                                    lhsT=self.est[b][:, P : 2 * P],
                                    rhs=self.vau[b][:, 0:392],
                                    start=True,
                                    stop=False,
                                )
                                nc.tensor.matmul(
                                    p_o[:, : H + 8],
                                    lhsT=self.est[b][:, 2 * P : 3 * P],
                                    rhs=self.vau[b][:, 400:792],
                                    start=False,
                                    stop=True,
                                )
                            self.po.append(p_o)

                def out_norms(self):
                    osb = self.osb
                    for b in range(2):
                        for tcc in range(2):
                            p_o = self.po[b * 2 + tcc]
                            rec = obp.tile([P, 1], F32, name="rec", tag="rec")
                            nc.vector.reciprocal(rec, p_o[:, H : H + 1])
                            dst = osb[:, tcc * 768 + b * 384 : tcc * 768 + (b + 1) * 384]
                            if tcc == 0:
                                nc.vector.tensor_scalar_mul(dst, p_o[:, :H], rec)
                            else:
                                nc.scalar.activation(
                                    dst, p_o[:, :H], COPY, scale=rec
                                )
                            nc.sync.dma_start(
                                out_d[self.b0 + b, tcc * P : (tcc + 1) * P, :],
                                dst,
                            )


            # pair 0's input DMA first on the GpSimd SWDGE queue (its
            # transposes gate the whole pipeline); weight loads follow
            pair0 = Pair(0)
            pair0.load()

            # Load weights: W[hc*128+p, c] -> wX_all[p, hc*384 + c], one DMA
            # each, rounding f32 -> bf16 in the DMA so the G matmuls run at
            # bf16 speed with fast weight load (G error contribution is well
            # inside the tolerance; verified in CoreSim)
            wq_s, wk_s, wv_s = [], [], []
            for lst, srcd, nm in (
                (wq_s, wq_d, "wq"),
                (wk_s, wk_d, "wk"),
                (wv_s, wv_d, "wv"),
            ):
                w_all = cpool.tile([P, CC * C], BF16, name=f"{nm}_all")
                nc.gpsimd.dma_start(
                    w_all.rearrange("p (hc c) -> p hc c", hc=CC),
                    srcd.rearrange("(hc p) c -> p hc c", hc=CC),
                )
                for hc in range(CC):
                    lst.append(w_all[:, hc * C : (hc + 1) * C])

            pair0.transpose()

            # G = (Wq.T @ Wk) * SCALE -> bf16  g_all[:, c1*384 + c2col]
            g_all = cpool.tile([P, CC * C], BF16, name="g_all")
            for c1 in range(CC):
                pg = pp.tile([P, 512], F32, name="pg", tag=("PST", "PO0", "PO1")[c1])
                for hc in range(CC):
                    nc.tensor.matmul(
                        pg[:, :C],
                        lhsT=wq_s[hc][:, c1 * P : (c1 + 1) * P],
                        rhs=wk_s[hc],
                        start=(hc == 0),
                        stop=(hc == CC - 1),
                    )
                nc.vector.tensor_scalar_mul(
                    g_all[:, c1 * C : (c1 + 1) * C], pg[:, :C], SCALE
                )

            # wvT_all[:, cc*384 + hc*128 + h'] bf16 (wv_s already bf16)
            pwt = pp.tile([P, 2048], BF16, name="pwt", tag="TV")
            for cc_ in range(CC):
                for hc in range(CC):
                    nc.tensor.transpose(
                        pwt[:, (cc_ * CC + hc) * P : (cc_ * CC + hc + 1) * P],
                        wv_s[hc][:, cc_ * P : (cc_ + 1) * P],
                        ident,
                    )
            wvT_all = cpool.tile([P, CC * H], BF16, name="wvT_all")
            nc.vector.tensor_copy(wvT_all, pwt[:, : CC * H])

            pairs = [pair0] + [Pair(pr) for pr in range(1, npair)]
            if npair > 1:
                pairs[1].load()  # prefetch
            prev = None
            for pr in range(npair):
                cur = pairs[pr]
                if pr > 0:
                    cur.transpose()
                if pr + 2 < npair:
                    # issue pair k+2's input DMA a full pair early so the
                    # transfer fully overlaps compute
                    pairs[pr + 2].load()
                if prev is not None:
                    # previous pair's output matmuls run while DVE drains
                    # this pair's transpose PSUM into xT
                    prev.out_mms()
                    prev.out_norms()
                cur.z2_mms()
                cur.vst()
                prev = cur
            prev.out_mms()
            prev.out_norms()

    nc.compile()
    return nc


_NC_CACHE = {}


def _get_nc(nb: int):
    if nb not in _NC_CACHE:
        _NC_CACHE[nb] = build_bass(nb)
    return _NC_CACHE[nb]


def kernel(x: np.ndarray, Wk: np.ndarray, Wq: np.ndarray, Wv: np.ndarray, **_):
    x = np.ascontiguousarray(x, dtype=np.float32)
    Wk = np.ascontiguousarray(Wk, dtype=np.float32)
    Wq = np.ascontiguousarray(Wq, dtype=np.float32)
    Wv = np.ascontiguousarray(Wv, dtype=np.float32)
    nb = x.shape[0] // NCORES
    nc = _get_nc(nb)
    in_maps = [
        {"x": x[i * nb : (i + 1) * nb], "Wk": Wk, "Wq": Wq, "Wv": Wv}
        for i in range(NCORES)
    ]
    res = run_bass_kernel_spmd(nc, in_maps, core_ids=list(range(NCORES)))
    return np.concatenate([r["out"] for r in res.results], axis=0)


if __name__ == "__main__":
    rng = np.random.default_rng(0)
    x = rng.standard_normal((B, T, C), dtype=np.float32)
    s = 1.0 / np.sqrt(C)
    Wk = rng.standard_normal((H, C), dtype=np.float32) * s
    Wq = rng.standard_normal((H, C), dtype=np.float32) * s
    Wv = rng.standard_normal((H, C), dtype=np.float32) * s
    out = kernel(x=x, Wk=Wk, Wq=Wq, Wv=Wv)
    print(out.shape, out.dtype)
